# revision 8
# baseline (speedup 1.0000x reference)
"""Causal multi-head attention on 8 TRN2 NeuronCores.

Sharding: core c -> (batch b = c // 2, head-half hh = c % 2).
Each core computes QKV for its 8 heads over the full sequence of its batch,
causal flash attention, and a partial out-projection using its 512 rows of
w_out. The host sums the two partials per batch (the "all-reduce" of the
tensor-parallel out projection).

Shapes (hardcoded): B=4, T=2048, D=1024, H=16, HD=64.
"""
import sys

for _p in ('/opt/trn_rl_repo', '/root/.axon_site/_ro/trn_rl_repo'):
    if _p not in sys.path:
        sys.path.insert(0, _p)

import numpy as np

B, T, D = 4, 2048, 1024
H, HD = 16, 64
HPC = H // 2          # heads per core = 8
DPC = HPC * HD        # out-dims per core = 512
N_CORES = 8

_nc_cache = {}


def _build_nc():
    import concourse.bacc as bacc
    import concourse.mybir as mybir
    from concourse.tile import TileContext

    F32 = mybir.dt.float32
    F32R = mybir.dt.float32r
    AF = mybir.ActivationFunctionType
    ALU = mybir.AluOpType

    CH = 256              # phase-1 token chunk
    QC = 512              # phase-2 query chunk
    NKB = T // 128        # 16 k-blocks
    NQC = T // QC         # 4 query chunks
    NCH = T // CH         # 8 phase-1 chunks
    NDT = D // 128        # 8 input-dim tiles
    VW = HPC * (HD + 1)   # V tile width = 520

    nc = bacc.Bacc('TRN2', target_bir_lowering=False, debug=False)
    xT_d = nc.dram_tensor('xT', [D, T], F32R, kind='ExternalInput')
    wq_d = nc.dram_tensor('wq', [D, DPC], F32R, kind='ExternalInput')
    wk_d = nc.dram_tensor('wk', [D, DPC], F32R, kind='ExternalInput')
    wv_d = nc.dram_tensor('wv', [D, DPC], F32R, kind='ExternalInput')
    wo_d = nc.dram_tensor('wo', [DPC, D], F32R, kind='ExternalInput')
    ones8_d = nc.dram_tensor('ones8', [128, HPC], F32R, kind='ExternalInput')
    onesc_d = nc.dram_tensor('onesc', [1, HD], F32R, kind='ExternalInput')
    sel2_d = nc.dram_tensor('sel2', [2, 128], F32R, kind='ExternalInput')
    po_d = nc.dram_tensor('po', [T, D], F32, kind='ExternalOutput')

    with nc.allow_low_precision(reason='fp32r matmuls by design'), \
            TileContext(nc) as tc:
        with (
            tc.tile_pool(name='kt', bufs=1) as kt_pool,
            tc.tile_pool(name='qt', bufs=1) as qt_pool,
            tc.tile_pool(name='vv', bufs=1) as v_pool,
            tc.tile_pool(name='ao', bufs=2) as ao_pool,
            tc.tile_pool(name='small', bufs=2) as sm_pool,
            tc.tile_pool(name='pt', bufs=5) as pt_pool,
            tc.tile_pool(name='osb', bufs=2) as osb_pool,
        ):
            # persistent SBUF tensors
            KT = [kt_pool.tile([128, T], F32R, tag=f'kt{j}', name=f'kt{j}') for j in range(4)]
            QT = [qt_pool.tile([128, T], F32R, tag=f'qt{j}', name=f'qt{j}') for j in range(4)]
            V = [v_pool.tile([128, VW], F32R, tag=f'v{t}', name=f'v{t}') for t in range(NKB)]
            for t in range(NKB):
                vt3 = V[t].rearrange('p (h c) -> p h c', c=HD + 1)
                nc.sync.dma_start(vt3[:, :, HD], ones8_d[:, :])

            # ---------------- Phase 1: projections ----------------
            with (
                tc.tile_pool(name='wqkv', bufs=1) as w_pool,
                tc.tile_pool(name='xs', bufs=2) as x_pool,
                tc.tile_pool(name='ps1', bufs=2, space='PSUM') as ps1,
            ):
                WQ = [w_pool.tile([128, DPC], F32R, tag=f'wq{d}', name=f'wqs{d}') for d in range(NDT)]
                WK = [w_pool.tile([128, DPC], F32R, tag=f'wk{d}', name=f'wks{d}') for d in range(NDT)]
                WV = [w_pool.tile([128, DPC], F32R, tag=f'wv{d}', name=f'wvs{d}') for d in range(NDT)]
                for d in range(NDT):
                    nc.sync.dma_start(WQ[d][:, :], wq_d[d*128:(d+1)*128, :])
                    nc.sync.dma_start(WK[d][:, :], wk_d[d*128:(d+1)*128, :])
                    nc.sync.dma_start(WV[d][:, :], wv_d[d*128:(d+1)*128, :])

                for c in range(NCH):
                    xs = [x_pool.tile([128, CH], F32R, tag=f'x{d}', name=f'xs{d}')
                          for d in range(NDT)]
                    for d in range(NDT):
                        nc.sync.dma_start(
                            xs[d][:, :], xT_d[d*128:(d+1)*128, c*CH:(c+1)*CH])
                    # KT / QT: out [128 dout, CH tok]
                    for w_tiles, dst in ((WK, KT), (WQ, QT)):
                        for j in range(4):
                            pp = ps1.tile([128, CH], F32, tag='p1')
                            for d in range(NDT):
                                nc.tensor.matmul(
                                    pp[:, :],
                                    lhsT=w_tiles[d][:, j*128:(j+1)*128],
                                    rhs=xs[d][:, :],
                                    start=(d == 0), stop=(d == NDT - 1))
                            nc.scalar.activation(
                                dst[j][:, c*CH:(c+1)*CH], pp[:, :], AF.Copy)
                    # V: out [128 tok, DPC dout]
                    for tt in range(CH // 128):
                        pv = ps1.tile([128, DPC], F32, tag='pv')
                        for d in range(NDT):
                            nc.tensor.matmul(
                                pv[:, :],
                                lhsT=xs[d][:, tt*128:(tt+1)*128],
                                rhs=WV[d][:, :],
                                start=(d == 0), stop=(d == NDT - 1))
                        vt = V[c*(CH // 128) + tt]
                        vt3 = vt.rearrange('p (h c) -> p h c', c=HD + 1)
                        nc.scalar.activation(
                            vt3[:, :, 0:HD],
                            pv.rearrange('p (h c) -> p h c', c=HD), AF.Copy)

            # ---------------- Phase 2 + 3: attention + out-proj ----------------
            with (
                tc.tile_pool(name='wo', bufs=1) as wo_pool,
                tc.tile_pool(name='ps_s', bufs=3, space='PSUM') as ps_s,
                tc.tile_pool(name='ps_ot', bufs=2, space='PSUM') as ps_ot,
                tc.tile_pool(name='ps_m', bufs=2, space='PSUM') as ps_m,
                tc.tile_pool(name='ps_pr', bufs=1, space='PSUM') as ps_pr,
            ):
                WO = [wo_pool.tile([128, D], F32R, tag=f'wo{d}', name=f'wos{d}') for d in range(4)]
                for d in range(4):
                    nc.sync.dma_start(WO[d][:, :], wo_d[d*128:(d+1)*128, :])

                for c in range(NQC):
                    q0 = c * QC
                    nkb = (q0 + QC) // 128      # causal k-blocks for this chunk
                    ao = [ao_pool.tile([128, QC], F32R, tag=f'ao{j}', name=f'ao{j}')
                          for j in range(4)]
                    for j in range(4):            # head pair (2j, 2j+1)
                        h0, h1 = 2*j, 2*j + 1
                        ot0 = ps_ot.tile([HD + 1, QC], F32, tag='ot', name='ot0')
                        ot1 = ps_ot.tile([HD + 1, QC], F32, tag='ot', name='ot1')
                        pend = None  # (kb, lo, pt0, pt1)
                        for kb in range(nkb):
                            lo = max(0, kb*128 - q0)
                            s0 = ps_s.tile([128, QC], F32, tag='s', name='s0')
                            s1 = ps_s.tile([128, QC], F32, tag='s', name='s1')
                            pt0 = pt_pool.tile([128, QC], F32R, tag='pt', name='pt0')
                            pt1 = pt_pool.tile([128, QC], F32R, tag='pt', name='pt1')
                            nc.tensor.matmul(
                                s0[:, lo:QC],
                                lhsT=KT[j][0:64, kb*128:(kb+1)*128],
                                rhs=QT[j][0:64, q0+lo:q0+QC],
                                start=True, stop=True)
                            nc.tensor.matmul(
                                s1[:, lo:QC],
                                lhsT=KT[j][64:128, kb*128:(kb+1)*128],
                                rhs=QT[j][64:128, q0+lo:q0+QC],
                                start=True, stop=True)
                            if pend is not None:
                                pkb, plo, ppt0, ppt1 = pend
                                nc.tensor.matmul(
                                    ot0[:, plo:QC],
                                    lhsT=V[pkb][:, (HD+1)*h0:(HD+1)*(h0+1)],
                                    rhs=ppt0[:, plo:QC],
                                    start=(pkb == 0), stop=False)
                                nc.tensor.matmul(
                                    ot1[:, plo:QC],
                                    lhsT=V[pkb][:, (HD+1)*h1:(HD+1)*(h1+1)],
                                    rhs=ppt1[:, plo:QC],
                                    start=(pkb == 0), stop=False)
                            nc.scalar.activation(
                                pt0[:, lo:QC], s0[:, lo:QC], AF.Exp)
                            nc.scalar.activation(
                                pt1[:, lo:QC], s1[:, lo:QC], AF.Exp)
                            if kb*128 >= q0:  # diagonal block: mask k > q
                                for ptx in (pt0, pt1):
                                    nc.gpsimd.affine_select(
                                        out=ptx[:, lo:lo+128],
                                        in_=ptx[:, lo:lo+128],
                                        compare_op=ALU.is_ge, fill=0.0, base=0,
                                        channel_multiplier=-1,
                                        pattern=[[1, 128]])
                            pend = (kb, lo, pt0, pt1)
                        pkb, plo, ppt0, ppt1 = pend
                        nc.tensor.matmul(
                            ot0[:, plo:QC],
                            lhsT=V[pkb][:, (HD+1)*h0:(HD+1)*(h0+1)],
                            rhs=ppt0[:, plo:QC],
                            start=(pkb == 0), stop=True)
                        nc.tensor.matmul(
                            ot1[:, plo:QC],
                            lhsT=V[pkb][:, (HD+1)*h1:(HD+1)*(h1+1)],
                            rhs=ppt1[:, plo:QC],
                            start=(pkb == 0), stop=True)
                        # normalize both heads of the pair
                        rp0 = sm_pool.tile([1, QC], F32, tag='rp0', bufs=1)
                        rp1 = sm_pool.tile([1, QC], F32, tag='rp1', bufs=1)
                        nc.vector.reciprocal(rp0[:, :], ot0[HD:HD+1, :])
                        nc.vector.reciprocal(rp1[:, :], ot1[HD:HD+1, :])
                        rbs0 = sm_pool.tile([HD, QC], F32, tag='rbs0', bufs=1)
                        rbs1 = sm_pool.tile([HD, QC], F32, tag='rbs1', bufs=1)
                        nc.gpsimd.partition_broadcast(rbs0[:, :], rp0[:, :])
                        nc.gpsimd.partition_broadcast(rbs1[:, :], rp1[:, :])
                        nc.vector.tensor_tensor(
                            out=ao[j][0:HD, :], in0=ot0[0:HD, :],
                            in1=rbs0[:, :], op=ALU.mult)
                        nc.vector.tensor_tensor(
                            out=ao[j][HD:128, :], in0=ot1[0:HD, :],
                            in1=rbs1[:, :], op=ALU.mult)
                    # fused partial out-projection for this q-chunk
                    for qt in range(QC // 128):
                        os = osb_pool.tile([128, D], F32, tag='os')
                        for half in range(2):
                            pj = ps_pr.tile([128, 512], F32, tag='pj')
                            for d in range(4):
                                nc.tensor.matmul(
                                    pj[:, :],
                                    lhsT=ao[d][:, qt*128:(qt+1)*128],
                                    rhs=WO[d][:, half*512:(half+1)*512],
                                    start=(d == 0), stop=(d == 3))
                            nc.vector.tensor_copy(
                                os[:, half*512:(half+1)*512], pj[:, :])
                        nc.sync.dma_start(
                            po_d[q0+qt*128:q0+(qt+1)*128, :], os[:, :])

    nc.compile()
    return nc


def _get_nc():
    if 'nc' not in _nc_cache:
        _nc_cache['nc'] = _build_nc()
    return _nc_cache['nc']


def kernel(x, w_qkv, w_out, _profile=False):
    from concourse.bass_utils import run_bass_kernel_spmd

    x = np.asarray(x, dtype=np.float32)
    w_qkv = np.asarray(w_qkv, dtype=np.float32)
    w_out = np.asarray(w_out, dtype=np.float32)

    nc = _get_nc()

    scale = np.float32(1.0 / np.sqrt(HD))
    ones8 = np.ones((128, HPC), np.float32)
    onesc = np.ones((1, HD), np.float32)
    sel2 = np.zeros((2, 128), np.float32)
    sel2[0, 0:HD] = 1.0
    sel2[1, HD:128] = 1.0
    in_maps = []
    for c in range(N_CORES):
        b, hh = c // 2, c % 2
        s, e = hh * DPC, (hh + 1) * DPC
        in_maps.append({
            'xT': np.ascontiguousarray(x[b].T),
            'wq': np.ascontiguousarray(w_qkv[:, s:e] * scale),
            'wk': np.ascontiguousarray(w_qkv[:, D+s:D+e]),
            'wv': np.ascontiguousarray(w_qkv[:, 2*D+s:2*D+e]),
            'wo': np.ascontiguousarray(w_out[s:e, :]),
            'ones8': ones8,
            'onesc': onesc,
            'sel2': sel2,
        })

    res = run_bass_kernel_spmd(nc, in_maps, core_ids=list(range(N_CORES)),
                               trace=_profile)
    out = np.empty((B, T, D), np.float32)
    for b in range(B):
        out[b] = res.results[2*b]['po'] + res.results[2*b+1]['po']
    if _profile:
        return out, res
    return out
